# revision 5
# baseline (speedup 1.0000x reference)
"""Llama4 MoE (T=4096 H=2048 I=1024 E=16 top-1) on 8 trn2 cores, expert-parallel.

Strategy:
  - Host computes the routing permutation (argmax of logits) and shards inputs:
    core c owns experts 2c, 2c+1 and receives their tokens (transposed,
    padded to 384/expert) plus its 512-token slice for the shared expert.
  - Device (SPMD, one program): router logits -> sigmoid weights -> scale
    gathered tokens -> expert GEMMs (bf16 in, fp32 accum) -> scatter rows
    into an AllToAll buffer blocked by destination core -> AllToAll ->
    indirect-gather + add to shared-expert output -> per-core [512, 2048]
    output shard. Host concatenates shards.
"""
import numpy as np

import concourse.bass as bass
import concourse.mybir as mybir
import concourse.tile as tile
from concourse import bacc
from concourse.bass_utils import run_bass_kernel_spmd
from concourse.masks import make_identity

T, H, I, E = 4096, 2048, 1024, 16
NCORES = 8
S = T // NCORES          # 512 tokens per output slice
EPC = E // NCORES        # 2 experts per core
CE = 384                 # per-expert token capacity (3 tiles of 128)
C = EPC * CE             # 768 gathered tokens per core
B = 96                   # AllToAll rows per (src,dst) block
NB = NCORES * B          # 768 rows in send/recv buffers
KT = H // 128            # 16 contraction tiles over H
ITI = I // 128           # 8 tiles over intermediate dim
MT_S = S // 128          # 4 token tiles per slice
MT_E = CE // 128         # 3 token tiles per expert
F32 = mybir.dt.float32
BF16 = mybir.dt.bfloat16
I32 = mybir.dt.int32

_CACHE = {}


def _build():
    nc = bacc.Bacc("TRN2", target_bir_lowering=False, debug=False,
                   enable_asserts=False, num_devices=NCORES)

    xgT = nc.dram_tensor("xgT", [H, C], F32, kind="ExternalInput").ap()
    xsT = nc.dram_tensor("xsT", [H, S], F32, kind="ExternalInput").ap()
    rw = nc.dram_tensor("rw", [H, E], F32, kind="ExternalInput").ap()
    ew1 = nc.dram_tensor("ew1", [EPC, H, I], F32, kind="ExternalInput").ap()
    ew3 = nc.dram_tensor("ew3", [EPC, H, I], F32, kind="ExternalInput").ap()
    ew2 = nc.dram_tensor("ew2", [EPC, I, H], F32, kind="ExternalInput").ap()
    sw1 = nc.dram_tensor("sw1", [H, I], F32, kind="ExternalInput").ap()
    sw3 = nc.dram_tensor("sw3", [H, I], F32, kind="ExternalInput").ap()
    sw2 = nc.dram_tensor("sw2", [I, H], F32, kind="ExternalInput").ap()
    sidx = nc.dram_tensor("sidx", [C, 1], I32, kind="ExternalInput").ap()
    gidx = nc.dram_tensor("gidx", [S, 1], I32, kind="ExternalInput").ap()
    out = nc.dram_tensor("out", [S, H], F32, kind="ExternalOutput").ap()

    with tile.TileContext(nc) as tc:
        with (
            tc.tile_pool(name="persist", bufs=1) as pp,
            tc.tile_pool(name="hpool", bufs=1) as hp,
            tc.tile_pool(name="ypool", bufs=3) as yp,
            tc.tile_pool(name="rpool", bufs=2) as rp,
            tc.tile_pool(name="stream", bufs=3) as sp,
            tc.tile_pool(name="wdpool", bufs=10) as wdp,
            tc.tile_pool(name="psum", bufs=1, space="PSUM") as psp,
            tc.tile_pool(name="dram", bufs=1, space="DRAM") as dp,
        ):
            send = dp.tile([NB, H], F32, tag="send")
            recv = dp.tile([NB, H], F32, tag="recv")

            # ---- load gathered tokens (bf16 cast) + indices + router w ----
            XG = pp.tile([128, KT * C], BF16, tag="xg")
            for k in range(KT):
                nc.gpsimd.dma_start(XG[:, k * C:(k + 1) * C],
                                    xgT[k * 128:(k + 1) * 128, :])
            RW = pp.tile([128, KT * E], BF16, tag="rw")
            rwt = rw.rearrange("(kt p) e -> kt p e", p=128)
            for k in range(KT):
                nc.gpsimd.dma_start(RW[:, k * E:(k + 1) * E], rwt[k])
            SIDX = pp.tile([128, C // 128], I32, tag="sidx")
            sxt = sidx.rearrange("(m p) one -> m p one", p=128)
            for m in range(C // 128):
                nc.sync.dma_start(SIDX[:, m:m + 1], sxt[m])
            GIDX = pp.tile([128, MT_S], I32, tag="gidx")
            gxt = gidx.rearrange("(m p) one -> m p one", p=128)
            for m in range(MT_S):
                nc.sync.dma_start(GIDX[:, m:m + 1], gxt[m])

            IDN = pp.tile([128, 128], F32, tag="idn")
            make_identity(nc, IDN[:])
            ONES = pp.tile([1, 128], BF16, tag="ones")
            nc.gpsimd.memset(ONES[:1, :], 1.0)

            # ---- router: logits -> max -> sigmoid -> broadcast over rows ----
            NM = C // 128  # 6 token tiles
            WSIG = pp.tile([128, NM], F32, tag="wsig")
            for m in range(NM):
                pl = psp.tile([128, E], F32, tag="pu0", space="PSUM")
                for k in range(KT):
                    nc.tensor.matmul(
                        pl[:], XG[:, k * C + m * 128:k * C + (m + 1) * 128],
                        RW[:, k * E:(k + 1) * E],
                        start=(k == 0), stop=(k == KT - 1))
                lmax = sp.tile([128, 1], F32, tag="lmax")
                nc.vector.reduce_max(lmax[:], pl[:], axis=mybir.AxisListType.X)
                nc.scalar.activation(WSIG[:, m:m + 1], lmax[:],
                                     mybir.ActivationFunctionType.Sigmoid)
            # transpose -> [NM, 128] (row m = weights of token tile m)
            pt = psp.tile([128, 128], F32, tag="pu1", space="PSUM")
            nc.tensor.transpose(pt[:NM, :], WSIG[:, :NM], IDN[:])
            WBC = pp.tile([128, C], BF16, tag="wbc")
            WR6 = pp.tile([NM, 128], BF16, tag="wr6")
            nc.vector.tensor_copy(WR6[:NM, :], pt[:NM, :])
            for m in range(NM):
                wrm = sp.tile([1, 128], BF16, tag="wrm")
                nc.sync.dma_start(wrm[:1, :], WR6[m:m + 1, :])
                pb = psp.tile([128, 128], F32, tag="pu2", space="PSUM")
                nc.tensor.matmul(pb[:], ONES[:1, :], wrm[:1, :],
                                 start=True, stop=True)
                nc.vector.tensor_copy(WBC[:, m * 128:(m + 1) * 128], pb[:])
            # scale gathered tokens by routing weight (columns are tokens)
            for k in range(KT):
                nc.vector.tensor_mul(XG[:, k * C:(k + 1) * C],
                                     XG[:, k * C:(k + 1) * C], WBC[:])

            # ---- gated MLP (up in 2 half-I passes on 4 psum banks;
            #      down streams w2 col-panels, sink consumes [128,512] psums) ----
            def gated_mlp(xtile, xoff, ntok, w1d, w3d, w2d, xstride, ysink):
                HH = []
                for mat, wd in ((0, w1d), (1, w3d)):
                    HT = hp.tile([128, ITI * ntok], BF16, tag=f"h{mat}_{ntok}")
                    for half in range(2):
                        pus = [psp.tile([128, ntok], F32, tag=f"pu{i}", name=f"pu{i}",
                                        space="PSUM") for i in range(4)]
                        for k in range(KT):
                            wp = sp.tile([128, 512], BF16, tag="wup")
                            nc.gpsimd.dma_start(
                                wp[:], wd[k * 128:(k + 1) * 128,
                                          half * 512:(half + 1) * 512])
                            for i in range(4):
                                nc.tensor.matmul(
                                    pus[i][:], wp[:, i * 128:(i + 1) * 128],
                                    xtile[:, k * xstride + xoff:
                                          k * xstride + xoff + ntok],
                                    start=(k == 0), stop=(k == KT - 1))
                        for i in range(4):
                            it = half * 4 + i
                            nc.vector.tensor_copy(
                                HT[:, it * ntok:(it + 1) * ntok], pus[i][:])
                    HH.append(HT)
                H1, H3 = HH
                nc.scalar.activation(H1[:], H1[:],
                                     mybir.ActivationFunctionType.Silu)
                nc.vector.tensor_mul(H1[:], H1[:], H3[:])
                nmt = ntok // 128
                for half in range(2):
                    wps = [wdp.tile([128, 1024], BF16, tag="wdn", name="wdn")
                           for _ in range(8)]
                    for k in range(8):
                        nc.gpsimd.dma_start(
                            wps[k][:], w2d[k * 128:(k + 1) * 128,
                                           half * 1024:(half + 1) * 1024])
                    for m in range(nmt):
                        for n2 in range(2):
                            pd = psp.tile([128, 512], F32, tag=f"pd{m % 3}",
                                          space="PSUM")
                            for k in range(8):
                                nc.tensor.matmul(
                                    pd[:],
                                    H1[:, k * ntok + m * 128:
                                       k * ntok + (m + 1) * 128],
                                    wps[k][:, n2 * 512:(n2 + 1) * 512],
                                    start=(k == 0), stop=(k == 7))
                            ysink(m, half * 1024 + n2 * 512, pd)

            # ---- routed experts: y rows -> indirect scatter to send buffer ----
            YT = {}

            def routed_sink(el):
                def sink(m, col, pd):
                    key = (el, m)
                    if key not in YT:
                        YT[key] = yp.tile([128, H], F32, tag="yrow", name="yrow")
                    nc.vector.tensor_copy(YT[key][:, col:col + 512], pd[:])
                    if col == H - 512:
                        gm = el * MT_E + m
                        nc.gpsimd.indirect_dma_start(
                            out=send[:],
                            out_offset=bass.IndirectOffsetOnAxis(
                                ap=SIDX[:, gm:gm + 1], axis=0),
                            in_=YT.pop(key)[:], in_offset=None)
                return sink

            for el in range(EPC):
                gated_mlp(XG, el * CE, CE, ew1[el], ew3[el], ew2[el],
                          C, routed_sink(el))

            # ---- shared expert on own token slice ----
            XS = pp.tile([128, KT * S], BF16, tag="xs")
            for k in range(KT):
                nc.gpsimd.dma_start(XS[:, k * S:(k + 1) * S],
                                    xsT[k * 128:(k + 1) * 128, :])
            YS = [pp.tile([128, H], F32, tag=f"ys{m}", name=f"ys{m}")
          for m in range(MT_S)]

            def shared_sink(m, col, pd):
                nc.vector.tensor_copy(YS[m][:, col:col + 512], pd[:])

            gated_mlp(XS, 0, S, sw1, sw3, sw2, S, shared_sink)

            # ---- combine: AllToAll + indirect gather + add + store ----
            nc.gpsimd.collective_compute(
                "AllToAll", mybir.AluOpType.bypass,
                replica_groups=[list(range(NCORES))],
                ins=[send[:].opt()], outs=[recv[:].opt()])
            for m in range(MT_S):
                rg = rp.tile([128, H], F32, tag="rg")
                nc.gpsimd.indirect_dma_start(
                    out=rg[:], out_offset=None, in_=recv[:],
                    in_offset=bass.IndirectOffsetOnAxis(
                        ap=GIDX[:, m:m + 1], axis=0))
                nc.vector.tensor_add(YS[m][:], YS[m][:], rg[:])
                nc.sync.dma_start(out[m * 128:(m + 1) * 128, :], YS[m][:])

    nc.compile()
    return nc


def kernel(hidden_states, router_w, shared_w1, shared_w3, shared_w2,
           expert_w1, expert_w3, expert_w2):
    hs = np.ascontiguousarray(np.asarray(hidden_states, dtype=np.float32))
    rw = np.ascontiguousarray(np.asarray(router_w, dtype=np.float32))
    logits = hs @ rw
    top = logits.argmax(1)
    toks = [np.flatnonzero(top == e) for e in range(E)]
    assert max(len(t) for t in toks) <= CE, "expert capacity exceeded"

    gidx_all = np.zeros(T, np.int64)
    in_maps = []
    for c in range(NCORES):
        xg = np.zeros((C, H), np.float32)
        send_idx = np.full((C,), c * B + B - 1, np.int64)  # pads -> dump row
        seq = np.zeros(NCORES, np.int64)
        for el in range(EPC):
            tk = toks[c * EPC + el]
            xg[el * CE:el * CE + len(tk)] = hs[tk]
            for i, t in enumerate(tk):
                d = t // S
                send_idx[el * CE + i] = d * B + seq[d]
                gidx_all[t] = c * B + seq[d]
                seq[d] += 1
        assert seq.max() <= B - 1, "A2A block capacity exceeded"
        in_maps.append({
            "xgT": np.ascontiguousarray(xg.T),
            "xsT": np.ascontiguousarray(hs[c * S:(c + 1) * S].T),
            "rw": rw,
            "ew1": np.ascontiguousarray(expert_w1[c * EPC:(c + 1) * EPC],
                                        dtype=np.float32),
            "ew3": np.ascontiguousarray(expert_w3[c * EPC:(c + 1) * EPC],
                                        dtype=np.float32),
            "ew2": np.ascontiguousarray(expert_w2[c * EPC:(c + 1) * EPC],
                                        dtype=np.float32),
            "sw1": np.ascontiguousarray(shared_w1, dtype=np.float32),
            "sw3": np.ascontiguousarray(shared_w3, dtype=np.float32),
            "sw2": np.ascontiguousarray(shared_w2, dtype=np.float32),
            "sidx": send_idx.astype(np.int32)[:, None],
            "gidx": np.zeros((S, 1), np.int32),
        })
    for c in range(NCORES):
        in_maps[c]["gidx"] = gidx_all[c * S:(c + 1) * S].astype(np.int32)[:, None]

    if "nc" not in _CACHE:
        _CACHE["nc"] = _build()
    res = run_bass_kernel_spmd(_CACHE["nc"], in_maps, list(range(NCORES)),
                               trace=False)
    return np.concatenate([res.results[c]["out"] for c in range(NCORES)], axis=0)
